# revision 18
# baseline (speedup 1.0000x reference)
"""nn_Attention4D (LeViT-style 4D attention with talking heads) on 8
axon-tunneled TRN2 NeuronCores.

Warm-call wall time is dominated by the axon host<->device tunnel
(~45 MB/s shared, ~45 ms RTT), not compute. Design:
  - x ships as bf16 (19.3 MB) via one sharded device_put per chunk;
    output ships as int8 + per-(sample,channel) f32 scales (9.8 MB).
  - All weights are folded on host (BN, SCALE, talking-head th1 into
    the Q projection) and cached device-resident across calls.
  - Compute is a Bass/Tile kernel (inlined below) run SPMD on all 8
    cores via bass_jit + shard_map (the same bass2jax/PJRT machinery
    run_bass_kernel_spmd uses under axon, but cached across calls);
    int8 quantization runs on-device as a jnp postlude.
  - 4 batch chunks of 32 pipeline host conversion, uploads, compute,
    and downloads against each other.
Fallbacks: XLA jnp path, then pure numpy.
"""
from contextlib import ExitStack

import numpy as np

B, DIM, RES, HEADS, KEY_DIM, ATTN_RATIO = 128, 384, 14, 8, 32, 4
D = ATTN_RATIO * KEY_DIM            # 128
DH = D * HEADS                      # 1024
N = RES * RES                       # 196
SCALE = KEY_DIM ** -0.5
NCORES = 8
BPC = B // NCORES                   # 16 samples per core
NCHUNK = 8                          # batch chunks per call
INT8_IN = True                      # ship x as int8 + per-(sample,ch) scales
QUANT_THREADS = 0                   # 0 = quantize inline on main thread
# per-core samples per chunk, earliest first; sums to BPC. Smaller final
# chunks halve the end-of-call fetch/drain tail; None = uniform CBPC.
SCHED = (2, 2, 2, 2, 2, 2, 1, 1, 1, 1)
CBPC = BPC // NCHUNK                # samples per core per chunk
NCH = [(0, 128), (128, 68)]         # n/m tiling of N=196

_cache = {}


# ---------------------------------------------------------------------------
# host-side weight folding
# ---------------------------------------------------------------------------

def _fold(w, b, s, t):
    # eval-mode BN folded into the preceding conv: y = (w@x + b)*s + t
    w = np.asarray(w, np.float32)
    b = np.asarray(b, np.float32)
    s = np.asarray(s, np.float32)
    t = np.asarray(t, np.float32)
    return (w * s[:, None]).astype(np.float32), (b * s + t).astype(np.float32)


def _prep_weights(q_w, q_b, q_scale, q_shift, k_w, k_b, k_scale, k_shift,
                  v_w, v_b, v_scale, v_shift, vl_w, vl_b, vl_scale, vl_shift,
                  th1_w, th1_b, th2_w, th2_b, proj_w, proj_b, proj_scale,
                  proj_shift, bias_seg, bias_idxs):
    qw, qb = _fold(q_w, q_b, q_scale, q_shift)
    kw, kb = _fold(k_w, k_b, k_scale, k_shift)
    vw, vb = _fold(v_w, v_b, v_scale, v_shift)
    vlw = (np.asarray(vl_w, np.float32)[:, 0] *
           np.asarray(vl_scale, np.float32)[:, None, None])
    vlb = (np.asarray(vl_b, np.float32) * np.asarray(vl_scale, np.float32) +
           np.asarray(vl_shift, np.float32))
    pw, pb = _fold(proj_w, proj_b, proj_scale, proj_shift)
    bias = np.asarray(bias_seg, np.float32)[:, np.asarray(bias_idxs)]  # [H,N,N]
    return (qw, qb, kw, kb, vw, vb, vlw, vlb,
            np.asarray(th1_w, np.float32), np.asarray(th1_b, np.float32),
            np.asarray(th2_w, np.float32), np.asarray(th2_b, np.float32),
            pw, pb, bias)


BASS_WEIGHT_ORDER = ("wqT", "wkT", "wvT", "wpT", "qb2", "kb2", "vb2", "vbf",
                     "bias1", "th2c", "th2bb", "vlb2", "pb2", "dwT")


def _prep_bass_weights(qw, qb, kw, kb, vw, vb, vlw, vlb,
                       th1w, th1b, th2w, th2b, pw, pb, bias):
    """Kernel-layout weight tensors. th1 (and SCALE) fold into the Q
    projection; the rel-pos bias is pre-mixed by th1 accordingly."""
    import ml_dtypes
    bf = ml_dtypes.bfloat16
    wq2 = (SCALE * th1w[:, :, None, None] *
           qw.reshape(1, 8, KEY_DIM, DIM)).reshape(2048, DIM)
    qb2v = (SCALE * th1w[:, :, None] * qb.reshape(1, 8, KEY_DIM)).reshape(2048)
    b1 = np.einsum('oi,inm->onm', th1w, bias) + th1b[:, None, None]
    dw = (vlw.reshape(8, 128, 9).transpose(1, 0, 2)[:, :, :, None] *
          np.eye(128, dtype=np.float32)[:, None, None, :])      # [k,8,9,m]
    return dict(
        wqT=np.ascontiguousarray(wq2.T).astype(bf),
        wkT=np.ascontiguousarray(kw.T).astype(bf),
        wvT=np.ascontiguousarray(vw.T).astype(bf),
        wpT=np.ascontiguousarray(pw.T).astype(bf),
        qb2=np.ascontiguousarray(qb2v.reshape(16, 128).T).astype(np.float32),
        kb2=np.ascontiguousarray(kb.reshape(2, 128).T).astype(np.float32),
        vb2=np.ascontiguousarray(vb.reshape(8, 128).T).astype(np.float32),
        vbf=vb.astype(np.float32),
        bias1=np.ascontiguousarray(
            b1.transpose(1, 0, 2).reshape(196, 1568)).astype(bf),
        th2c=th2w.reshape(64).astype(np.float32),
        th2bb=th2b.astype(np.float32),
        vlb2=np.ascontiguousarray(vlb.reshape(8, 128).T).astype(np.float32),
        pb2=np.ascontiguousarray(pb.reshape(3, 128).T).astype(np.float32),
        dwT=np.ascontiguousarray(dw.reshape(128, 72 * 128)).astype(bf),
    )


# ---------------------------------------------------------------------------
# Bass/Tile kernel: one core, `bpc` samples
# ---------------------------------------------------------------------------

def _build_bass_kernel(bpc):
    import concourse.bass as bass
    import concourse.tile as tile
    from concourse import mybir
    from concourse.bass2jax import bass_jit
    from concourse.masks import make_identity

    F32 = mybir.dt.float32
    BF16 = mybir.dt.bfloat16

    @bass_jit
    def attn_kernel(nc: bass.Bass, x, wqT, wkT, wvT, wpT, qb2, kb2, vb2,
                    vbf, bias1, th2c, th2bb, vlb2, pb2, dwT):
        y = nc.dram_tensor("y", [bpc, DIM, N], BF16, kind="ExternalOutput")
        with tile.TileContext(nc) as tc, ExitStack() as ctx:
            wp_ = ctx.enter_context(tc.tile_pool(name="wts", bufs=1))
            xin = ctx.enter_context(tc.tile_pool(name="xin", bufs=3))
            act = ctx.enter_context(tc.tile_pool(name="act", bufs=2))
            sml = ctx.enter_context(tc.tile_pool(name="sml", bufs=3))
            ps_ = ctx.enter_context(tc.tile_pool(name="ps", bufs=5, space="PSUM"))
            pst_ = ctx.enter_context(tc.tile_pool(name="pst", bufs=3, space="PSUM"))

            # ---- weights -> SBUF (once per exec) ----
            wq_sb = wp_.tile([128, 3 * 2048], BF16)
            wk_sb = wp_.tile([128, 3 * 256], BF16)
            wv_sb = wp_.tile([128, 3 * 1024], BF16)
            for kc in range(3):
                nc.sync.dma_start(out=wq_sb[:, kc * 2048:(kc + 1) * 2048],
                                  in_=wqT[kc * 128:(kc + 1) * 128, :])
                nc.sync.dma_start(out=wk_sb[:, kc * 256:(kc + 1) * 256],
                                  in_=wkT[kc * 128:(kc + 1) * 128, :])
                nc.sync.dma_start(out=wv_sb[:, kc * 1024:(kc + 1) * 1024],
                                  in_=wvT[kc * 128:(kc + 1) * 128, :])
            wp_sb = wp_.tile([128, 8 * 384], BF16)
            for dc in range(8):
                nc.sync.dma_start(out=wp_sb[:, dc * 384:(dc + 1) * 384],
                                  in_=wpT[dc * 128:(dc + 1) * 128, :])
            b1_sb = []
            for ic, (nsl, nn) in enumerate(NCH):
                t = wp_.tile([128, 1568], BF16, tag=f"b1_{ic}")
                nc.sync.dma_start(out=t[:nn, :], in_=bias1[nsl:nsl + nn, :])
                b1_sb.append(t)
            dw_sb = wp_.tile([128, 72 * 128], BF16)
            nc.sync.dma_start(out=dw_sb, in_=dwT[:, :])
            vbf_sb = wp_.tile([128, 1024], F32)
            nc.sync.dma_start(out=vbf_sb, in_=vbf[None, :].to_broadcast((128, 1024)))
            th2c_sb = wp_.tile([128, 64], F32)
            nc.sync.dma_start(out=th2c_sb, in_=th2c[None, :].to_broadcast((128, 64)))
            th2b_sb = wp_.tile([128, 8], F32)
            nc.sync.dma_start(out=th2b_sb, in_=th2bb[None, :].to_broadcast((128, 8)))
            qb_sb = wp_.tile([128, 16], F32)
            nc.sync.dma_start(out=qb_sb, in_=qb2[:, :])
            kb_sb = wp_.tile([128, 2], F32)
            nc.sync.dma_start(out=kb_sb, in_=kb2[:, :])
            vb_sb = wp_.tile([128, 8], F32)
            nc.sync.dma_start(out=vb_sb, in_=vb2[:, :])
            vlb_sb = wp_.tile([128, 8], F32)
            nc.sync.dma_start(out=vlb_sb, in_=vlb2[:, :])
            pb_sb = wp_.tile([128, 3], F32)
            nc.sync.dma_start(out=pb_sb, in_=pb2[:, :])
            ident = wp_.tile([128, 128], BF16)
            make_identity(nc, ident)

            AL = mybir.AluOpType
            AF = mybir.ActivationFunctionType

            for s in range(bpc):
                x_sb = xin.tile([128, 3, 196], BF16, tag="x")
                for kc in range(3):
                    nc.sync.dma_start(out=x_sb[:, kc, :],
                                      in_=x[s, kc * 128:(kc + 1) * 128, :])
                # ---- q' projection (th1+SCALE prefolded): [2048, n] ----
                q_sb = act.tile([128, 16 * 196], BF16, tag="q")
                for mo in range(16):
                    ps = ps_.tile([128, 512], F32, tag="mm")
                    pq = ps[:, :196]
                    for kc in range(3):
                        nc.tensor.matmul(
                            pq, wq_sb[:, kc * 2048 + mo * 128:kc * 2048 + (mo + 1) * 128],
                            x_sb[:, kc, :], start=(kc == 0), stop=(kc == 2))
                    nc.vector.tensor_scalar_add(
                        q_sb[:, mo * 196:(mo + 1) * 196], pq, qb_sb[:, mo:mo + 1])
                # ---- k projection: [256, m] ----
                k_sb = act.tile([128, 2 * 196], BF16, tag="k")
                for mo in range(2):
                    ps = ps_.tile([128, 512], F32, tag="mm")
                    pk = ps[:, :196]
                    for kc in range(3):
                        nc.tensor.matmul(
                            pk, wk_sb[:, kc * 256 + mo * 128:kc * 256 + (mo + 1) * 128],
                            x_sb[:, kc, :], start=(kc == 0), stop=(kc == 2))
                    nc.vector.tensor_scalar_add(
                        k_sb[:, mo * 196:(mo + 1) * 196], pk, kb_sb[:, mo:mo + 1])
                # ---- vT: [m(128+68), dh1024] (attention V incl bias) ----
                vT_sb = act.tile([128, 2 * 1024], BF16, tag="vT")
                for mc, (msl, mm) in enumerate(NCH):
                    for dsv in range(2):
                        ps = ps_.tile([128, 512], F32, tag="mm")
                        for kc in range(3):
                            nc.tensor.matmul(
                                ps[:mm, :], x_sb[:, kc, msl:msl + mm],
                                wv_sb[:, kc * 1024 + dsv * 512:kc * 1024 + (dsv + 1) * 512],
                                start=(kc == 0), stop=(kc == 2))
                        nc.vector.tensor_tensor(
                            vT_sb[:mm, mc * 1024 + dsv * 512:mc * 1024 + (dsv + 1) * 512],
                            ps[:mm, :], vbf_sb[:mm, dsv * 512:(dsv + 1) * 512], AL.add)
                # ---- v4 -> zero-padded vp tiles [d, 16x16] (bias here) ----
                vp_sb = act.tile([128, 8, 16, 16], BF16, tag="vp")
                for dc in range(8):
                    ps = ps_.tile([128, 512], F32, tag="mm")
                    pv = ps[:, :196]
                    for kc in range(3):
                        nc.tensor.matmul(
                            pv, wv_sb[:, kc * 1024 + dc * 128:kc * 1024 + (dc + 1) * 128],
                            x_sb[:, kc, :], start=(kc == 0), stop=(kc == 2))
                    nc.vector.memset(vp_sb[:, dc, :, :], 0.0)
                    nc.vector.tensor_scalar_add(
                        vp_sb[:, dc, 1:15, 1:15],
                        pv.rearrange("p (h w) -> p h w", h=14), vb_sb[:, dc:dc + 1])
                # ---- logits (incl bias1 via identity matmul), exp, sums ----
                exp_sb = act.tile([128, 2 * 1568], BF16, tag="exp")
                S_sb = sml.tile([128, 16], F32, tag="S")
                R_sb = sml.tile([128, 16], F32, tag="R")
                for o in range(8):
                    for ic, (nsl, nn) in enumerate(NCH):
                        ps = ps_.tile([128, 512], F32, tag="mm")
                        pl = ps[:nn, :196]
                        nc.tensor.matmul(pl, ident[:nn, :nn],
                                         b1_sb[ic][:nn, o * 196:(o + 1) * 196],
                                         start=True, stop=False)
                        for kc in range(2):
                            nc.tensor.matmul(
                                pl,
                                q_sb[:, (2 * o + kc) * 196 + nsl:(2 * o + kc) * 196 + nsl + nn],
                                k_sb[:, kc * 196:(kc + 1) * 196],
                                start=False, stop=(kc == 1))
                        nc.scalar.activation(
                            exp_sb[:nn, ic * 1568 + o * 196:ic * 1568 + (o + 1) * 196],
                            pl, AF.Exp,
                            accum_out=S_sb[:nn, ic * 8 + o:ic * 8 + o + 1])
                for ic, (nsl, nn) in enumerate(NCH):
                    nc.vector.reciprocal(R_sb[:nn, ic * 8:(ic + 1) * 8],
                                         S_sb[:nn, ic * 8:(ic + 1) * 8])
                # ---- normalize ----
                pn_sb = act.tile([128, 2 * 1568], BF16, tag="pn")
                for o in range(8):
                    for ic, (nsl, nn) in enumerate(NCH):
                        sl = slice(ic * 1568 + o * 196, ic * 1568 + (o + 1) * 196)
                        nc.vector.tensor_scalar_mul(
                            pn_sb[:nn, sl], exp_sb[:nn, sl],
                            R_sb[:nn, ic * 8 + o:ic * 8 + o + 1])
                # ---- per head: th2 mix (FMA chain), transpose, out ----
                xr_sb = act.tile([128, 8 * 196], BF16, tag="xr")
                for h in range(8):
                    ph = sml.tile([128, 2 * 196], BF16, tag="ph")
                    tmp = sml.tile([128, 2 * 2 * 196], BF16, tag="mixtmp")
                    for ic, (nsl, nn) in enumerate(NCH):
                        acc = [tmp[:nn, (2 * ic) * 196:(2 * ic + 1) * 196],
                               tmp[:nn, (2 * ic + 1) * 196:(2 * ic + 2) * 196]]
                        for o in range(8):
                            pno = pn_sb[:nn, ic * 1568 + o * 196:ic * 1568 + (o + 1) * 196]
                            dst = (ph[:nn, ic * 196:(ic + 1) * 196] if o == 7
                                   else acc[(o + 1) % 2])
                            if o == 0:
                                nc.vector.tensor_scalar(
                                    dst, pno, th2c_sb[:nn, h * 8:h * 8 + 1],
                                    th2b_sb[:nn, h:h + 1], AL.mult, AL.add)
                            else:
                                nc.vector.scalar_tensor_tensor(
                                    dst, pno, th2c_sb[:nn, h * 8 + o:h * 8 + o + 1],
                                    acc[o % 2], AL.mult, AL.add)
                    pt_sb = sml.tile([128, 2 * 196], BF16, tag="pt")
                    for mc, (msl, mm) in enumerate(NCH):
                        pst = pst_.tile([128, 512], BF16, tag="mmt")
                        for ic, (nsl, nn) in enumerate(NCH):
                            nc.tensor.transpose(
                                pst[:mm, nsl:nsl + nn],
                                ph[:nn, ic * 196 + msl:ic * 196 + msl + mm],
                                ident[:nn, :nn])
                        nc.scalar.copy(pt_sb[:mm, mc * 196:(mc + 1) * 196],
                                       pst[:mm, :196])
                    # out^T_h[d, n]: attention-out + 9 depthwise taps in
                    # one PSUM accumulation group, then fused ReLU+bias
                    pso = ps_.tile([128, 512], F32, tag="mm")
                    po = pso[:, :196]
                    for mc, (msl, mm) in enumerate(NCH):
                        nc.tensor.matmul(
                            po, vT_sb[:mm, mc * 1024 + h * 128:mc * 1024 + (h + 1) * 128],
                            pt_sb[:mm, mc * 196:(mc + 1) * 196],
                            start=(mc == 0), stop=False)
                    for t in range(9):
                        dy, dx = t // 3, t % 3
                        nc.tensor.matmul(
                            po, dw_sb[:, (h * 9 + t) * 128:(h * 9 + t + 1) * 128],
                            vp_sb[:, h, dy:dy + 14, dx:dx + 14],
                            start=False, stop=(t == 8))
                    nc.scalar.activation(xr_sb[:, h * 196:(h + 1) * 196], po,
                                         AF.Relu, bias=vlb_sb[:, h:h + 1])
                # ---- output projection ----
                y_sb = xin.tile([128, 3 * 196], BF16, tag="y")
                for oc in range(3):
                    ps = ps_.tile([128, 512], F32, tag="mm")
                    py = ps[:, :196]
                    for dc in range(8):
                        nc.tensor.matmul(
                            py, wp_sb[:, dc * 384 + oc * 128:dc * 384 + (oc + 1) * 128],
                            xr_sb[:, dc * 196:(dc + 1) * 196],
                            start=(dc == 0), stop=(dc == 7))
                    nc.vector.tensor_scalar_add(
                        y_sb[:, oc * 196:(oc + 1) * 196], py, pb_sb[:, oc:oc + 1])
                    nc.sync.dma_start(out=y[s, oc * 128:(oc + 1) * 128, :],
                                      in_=y_sb[:, oc * 196:(oc + 1) * 196])
        return (y,)

    return attn_kernel


# ---------------------------------------------------------------------------
# jnp compute fallback (shard body)
# ---------------------------------------------------------------------------

def _block(x, q_w, q_b, k_w, k_b, v_w, v_b, vl_w, vl_b,
           th1_w, th1_b, th2_w, th2_b, proj_w, proj_b, bias):
    import jax
    import jax.numpy as jnp
    b = x.shape[0]
    xf = x.astype(jnp.float32)
    q = jnp.einsum('oc,bcn->bon', q_w, xf) + q_b[:, None]
    k = jnp.einsum('oc,bcn->bon', k_w, xf) + k_b[:, None]
    v = jnp.einsum('oc,bcn->bon', v_w, xf) + v_b[:, None]
    v4 = v.reshape(b, DH, RES, RES)
    vp = jnp.pad(v4, ((0, 0), (0, 0), (1, 1), (1, 1)))
    vloc = vl_b[None, :, None, None]
    for dy in range(3):
        for dx in range(3):
            vloc = vloc + vl_w[:, dy, dx][None, :, None, None] * \
                vp[:, :, dy:dy + RES, dx:dx + RES]
    qh = q.reshape(b, HEADS, KEY_DIM, N)
    kh = k.reshape(b, HEADS, KEY_DIM, N)
    attn = jnp.einsum('bhcn,bhcm->bhnm', qh, kh) * SCALE + bias[None]
    attn = jnp.einsum('oi,binm->bonm', th1_w, attn) + th1_b[None, :, None, None]
    attn = jax.nn.softmax(attn, axis=-1)
    attn = jnp.einsum('oi,binm->bonm', th2_w, attn) + th2_b[None, :, None, None]
    vh = v.reshape(b, HEADS, D, N)
    out = jnp.einsum('bhnm,bhdm->bhdn', attn, vh)
    x_out = jax.nn.relu(out.reshape(b, DH, N) + vloc.reshape(b, DH, N))
    y = jnp.einsum('oc,bcn->bon', proj_w, x_out) + proj_b[:, None]
    return y.astype(jnp.bfloat16)


def _block_np(x, q_w, q_b, k_w, k_b, v_w, v_b, vl_w, vl_b,
              th1_w, th1_b, th2_w, th2_b, proj_w, proj_b, bias):
    b = x.shape[0]
    xf = x.reshape(b, DIM, N)
    q = np.einsum('oc,bcn->bon', q_w, xf) + q_b[:, None]
    k = np.einsum('oc,bcn->bon', k_w, xf) + k_b[:, None]
    v = np.einsum('oc,bcn->bon', v_w, xf) + v_b[:, None]
    v4 = v.reshape(b, DH, RES, RES)
    vp = np.pad(v4, ((0, 0), (0, 0), (1, 1), (1, 1)))
    vloc = np.broadcast_to(vl_b[None, :, None, None], v4.shape).copy()
    for dy in range(3):
        for dx in range(3):
            vloc += vl_w[:, dy, dx][None, :, None, None] * \
                vp[:, :, dy:dy + RES, dx:dx + RES]
    qh = q.reshape(b, HEADS, KEY_DIM, N)
    kh = k.reshape(b, HEADS, KEY_DIM, N)
    attn = np.einsum('bhcn,bhcm->bhnm', qh, kh) * SCALE + bias[None]
    attn = np.einsum('oi,binm->bonm', th1_w, attn) + th1_b[None, :, None, None]
    attn = attn - attn.max(-1, keepdims=True)
    np.exp(attn, out=attn)
    attn /= attn.sum(-1, keepdims=True)
    attn = np.einsum('oi,binm->bonm', th2_w, attn) + th2_b[None, :, None, None]
    vh = v.reshape(b, HEADS, D, N)
    out = np.einsum('bhnm,bhdm->bhdn', attn, vh)
    x_out = np.maximum(out.reshape(b, DH, RES, RES) + vloc, 0.0)
    y = np.einsum('oc,bcn->bon', proj_w, x_out.reshape(b, DH, N)) + proj_b[:, None]
    return y.reshape(b, DIM, RES, RES).astype(np.float32)


# ---------------------------------------------------------------------------
# dispatch
# ---------------------------------------------------------------------------

def kernel(x, **kw):
    x = np.asarray(x, np.float32)
    wargs = _prep_weights(**kw)
    try:
        return _run_bass(x, wargs)
    except Exception:
        import traceback
        traceback.print_exc()
        try:
            return _run_device(x, wargs)
        except Exception:
            traceback.print_exc()
            return _block_np(x, *wargs)


def _jax_setup():
    import os
    os.environ.setdefault("JAX_COMPILATION_CACHE_DIR", "/tmp/jax_comp_cache")
    import jax
    jax.config.update("jax_compilation_cache_dir",
                      os.environ["JAX_COMPILATION_CACHE_DIR"])
    jax.config.update("jax_persistent_cache_min_entry_size_bytes", -1)
    jax.config.update("jax_persistent_cache_min_compile_time_secs", 0)
    from jax.sharding import Mesh, PartitionSpec as P, NamedSharding
    devs = jax.devices()[:NCORES]
    mesh = Mesh(np.asarray(devs), ("b",))
    return jax, mesh, P, NamedSharding


def _run_bass(x, wargs):
    import ml_dtypes
    sched = tuple(SCHED) if (INT8_IN and SCHED) else (CBPC,) * NCHUNK
    assert sum(sched) == BPC
    fp = ("bass", sched) + tuple(float(a.sum()) for a in wargs)
    if _cache.get("bass_fp") != fp:
        jax, mesh, P, NamedSharding = _jax_setup()
        import jax.numpy as jnp
        bw = _prep_bass_weights(*wargs)
        sh_r = NamedSharding(mesh, P())
        sh_b = NamedSharding(mesh, P("b"))
        wdev = tuple(jax.device_put(bw[k], sh_r) for k in BASS_WEIGHT_ORDER)
        nw = len(BASS_WEIGHT_ORDER)

        def _quant_body(yb):
            yf = yb.astype(jnp.float32)
            am = jnp.max(jnp.abs(yf), axis=-1) + 1e-8
            sc = am / 127.0
            yq = jnp.round(yf / sc[:, :, None]).astype(jnp.int8)
            return yq, sc

        def _dequant_body(xq, xsc):
            return (xq.astype(jnp.float32) *
                    xsc[:, :, None]).astype(jnp.bfloat16)

        # one (bass, quant, dequant) jit triple per distinct chunk size.
        # The neuronx_cc hook requires the bass_exec custom call to be
        # the ONLY op in its module, so quant/dequant are separate jits.
        fns = {}
        for bpc in sorted(set(sched)):
            kern = _build_bass_kernel(bpc)
            f = jax.jit(jax.shard_map(
                lambda xb, *w, _k=kern: _k(xb, *w)[0], mesh=mesh,
                in_specs=(P("b"),) + (P(),) * nw,
                out_specs=P("b"), check_vma=False))
            fq = jax.jit(jax.shard_map(
                _quant_body, mesh=mesh, in_specs=(P("b"),),
                out_specs=(P("b"), P("b")), check_vma=False))
            fdq = jax.jit(jax.shard_map(
                _dequant_body, mesh=mesh, in_specs=(P("b"), P("b")),
                out_specs=P("b"), check_vma=False))
            # warm so timed calls skip tracing/compiling
            zx = np.zeros((NCORES * bpc, DIM, N), ml_dtypes.bfloat16)
            if INT8_IN:
                zq, zs = fq(f(fdq(
                    jax.device_put(zx.astype(np.int8), sh_b),
                    jax.device_put(
                        np.ones((NCORES * bpc, DIM), np.float32),
                        sh_b)), *wdev))
            else:
                zq, zs = fq(f(jax.device_put(zx, sh_b), *wdev))
            zq.block_until_ready()
            fns[bpc] = (f, fq, fdq)
        _cache.update(bass_fns=fns, bass_w=wdev, bass_fp=fp, bass_sh=sh_b)

    fns, wdev, sh_b = _cache["bass_fns"], _cache["bass_w"], _cache["bass_sh"]
    import jax
    xr = x.reshape(NCORES, BPC, DIM, N)
    offs = [0]
    for bpc in sched:
        offs.append(offs[-1] + bpc)

    def _quant_chunk(c):
        xc = np.ascontiguousarray(
            xr[:, offs[c]:offs[c + 1]]).reshape(-1, DIM, N)
        am = np.abs(xc).max(axis=-1) + 1e-8
        sc = (am / 127.0).astype(np.float32)
        np.rint(xc * (1.0 / sc)[:, :, None], out=xc)
        return xc.astype(np.int8), sc

    hs = []
    if INT8_IN:
        if QUANT_THREADS:
            from concurrent.futures import ThreadPoolExecutor
            ex = _cache.get("ex")
            if ex is None:
                ex = _cache["ex"] = ThreadPoolExecutor(QUANT_THREADS)
            futs = [ex.submit(_quant_chunk, c) for c in range(len(sched))]
            chunks = (futs[c].result() for c in range(len(sched)))
        else:
            chunks = (_quant_chunk(c) for c in range(len(sched)))
        for bpc, (xq, xsc) in zip(sched, chunks):
            f, fq, fdq = fns[bpc]
            xb = fdq(jax.device_put(xq, sh_b), jax.device_put(xsc, sh_b))
            yq, sc = fq(f(xb, *wdev))
            yq.copy_to_host_async()
            sc.copy_to_host_async()
            hs.append((yq, sc))
    else:
        for c, bpc in enumerate(sched):
            f, fq, fdq = fns[bpc]
            xc = np.ascontiguousarray(
                xr[:, offs[c]:offs[c + 1]]).reshape(-1, DIM, N)
            yq, sc = fq(f(jax.device_put(
                xc.astype(ml_dtypes.bfloat16), sh_b), *wdev))
            yq.copy_to_host_async()
            sc.copy_to_host_async()
            hs.append((yq, sc))
    out = np.empty((NCORES, BPC, DIM, N), np.float32)
    for c, (yq, sc) in enumerate(hs):
        yqh = np.asarray(yq).astype(np.float32)
        sch = np.asarray(sc)
        out[:, offs[c]:offs[c + 1]] = \
            (yqh * sch[:, :, None]).reshape(NCORES, sched[c], DIM, N)
    return out.reshape(B, DIM, RES, RES)


def _run_device(x, wargs):
    import ml_dtypes
    fp = ("jnp",) + tuple(float(a.sum()) for a in wargs)
    if _cache.get("fp") != fp:
        jax, mesh, P, NamedSharding = _jax_setup()
        f = jax.jit(jax.shard_map(
            _block, mesh=mesh,
            in_specs=(P("b"),) + (P(),) * len(wargs),
            out_specs=P("b"), check_vma=False))
        sh_r = NamedSharding(mesh, P())
        wdev = tuple(jax.device_put(w, sh_r) for w in wargs)
        _cache.update(f=f, wdev=wdev, fp=fp,
                      sh_b=NamedSharding(mesh, P("b")))
    import jax
    xb = x.reshape(B, DIM, N).astype(ml_dtypes.bfloat16)
    xd = jax.device_put(xb, _cache["sh_b"])
    y = _cache["f"](xd, *_cache["wdev"])
    return np.asarray(y).astype(np.float32).reshape(B, DIM, RES, RES)


# revision 19
# speedup vs baseline: 1.0298x; 1.0298x over previous
"""nn_Attention4D (LeViT-style 4D attention with talking heads) on 8
axon-tunneled TRN2 NeuronCores.

Warm-call wall time is dominated by the axon host<->device tunnel
(~45 MB/s shared, ~45 ms RTT), not compute. Design:
  - x ships as bf16 (19.3 MB) via one sharded device_put per chunk;
    output ships as int8 + per-(sample,channel) f32 scales (9.8 MB).
  - All weights are folded on host (BN, SCALE, talking-head th1 into
    the Q projection) and cached device-resident across calls.
  - Compute is a Bass/Tile kernel (inlined below) run SPMD on all 8
    cores via bass_jit + shard_map (the same bass2jax/PJRT machinery
    run_bass_kernel_spmd uses under axon, but cached across calls);
    int8 quantization runs on-device as a jnp postlude.
  - 4 batch chunks of 32 pipeline host conversion, uploads, compute,
    and downloads against each other.
Fallbacks: XLA jnp path, then pure numpy.
"""
from contextlib import ExitStack

import numpy as np

B, DIM, RES, HEADS, KEY_DIM, ATTN_RATIO = 128, 384, 14, 8, 32, 4
D = ATTN_RATIO * KEY_DIM            # 128
DH = D * HEADS                      # 1024
N = RES * RES                       # 196
SCALE = KEY_DIM ** -0.5
NCORES = 8
BPC = B // NCORES                   # 16 samples per core
NCHUNK = 8                          # batch chunks per call
INT8_IN = True                      # ship x as int8 + per-(sample,ch) scales
QUANT_THREADS = 0                   # 0 = quantize inline on main thread
# per-core samples per chunk, earliest first; sums to BPC. Smaller final
# chunks halve the end-of-call fetch/drain tail; None = uniform CBPC.
SCHED = (1, 2, 2, 2, 2, 2, 2, 1, 1, 1)
CBPC = BPC // NCHUNK                # samples per core per chunk
NCH = [(0, 128), (128, 68)]         # n/m tiling of N=196

_cache = {}


# ---------------------------------------------------------------------------
# host-side weight folding
# ---------------------------------------------------------------------------

def _fold(w, b, s, t):
    # eval-mode BN folded into the preceding conv: y = (w@x + b)*s + t
    w = np.asarray(w, np.float32)
    b = np.asarray(b, np.float32)
    s = np.asarray(s, np.float32)
    t = np.asarray(t, np.float32)
    return (w * s[:, None]).astype(np.float32), (b * s + t).astype(np.float32)


def _prep_weights(q_w, q_b, q_scale, q_shift, k_w, k_b, k_scale, k_shift,
                  v_w, v_b, v_scale, v_shift, vl_w, vl_b, vl_scale, vl_shift,
                  th1_w, th1_b, th2_w, th2_b, proj_w, proj_b, proj_scale,
                  proj_shift, bias_seg, bias_idxs):
    qw, qb = _fold(q_w, q_b, q_scale, q_shift)
    kw, kb = _fold(k_w, k_b, k_scale, k_shift)
    vw, vb = _fold(v_w, v_b, v_scale, v_shift)
    vlw = (np.asarray(vl_w, np.float32)[:, 0] *
           np.asarray(vl_scale, np.float32)[:, None, None])
    vlb = (np.asarray(vl_b, np.float32) * np.asarray(vl_scale, np.float32) +
           np.asarray(vl_shift, np.float32))
    pw, pb = _fold(proj_w, proj_b, proj_scale, proj_shift)
    bias = np.asarray(bias_seg, np.float32)[:, np.asarray(bias_idxs)]  # [H,N,N]
    return (qw, qb, kw, kb, vw, vb, vlw, vlb,
            np.asarray(th1_w, np.float32), np.asarray(th1_b, np.float32),
            np.asarray(th2_w, np.float32), np.asarray(th2_b, np.float32),
            pw, pb, bias)


BASS_WEIGHT_ORDER = ("wqT", "wkT", "wvT", "wpT", "qb2", "kb2", "vb2", "vbf",
                     "bias1", "th2c", "th2bb", "vlb2", "pb2", "dwT")


def _prep_bass_weights(qw, qb, kw, kb, vw, vb, vlw, vlb,
                       th1w, th1b, th2w, th2b, pw, pb, bias):
    """Kernel-layout weight tensors. th1 (and SCALE) fold into the Q
    projection; the rel-pos bias is pre-mixed by th1 accordingly."""
    import ml_dtypes
    bf = ml_dtypes.bfloat16
    wq2 = (SCALE * th1w[:, :, None, None] *
           qw.reshape(1, 8, KEY_DIM, DIM)).reshape(2048, DIM)
    qb2v = (SCALE * th1w[:, :, None] * qb.reshape(1, 8, KEY_DIM)).reshape(2048)
    b1 = np.einsum('oi,inm->onm', th1w, bias) + th1b[:, None, None]
    dw = (vlw.reshape(8, 128, 9).transpose(1, 0, 2)[:, :, :, None] *
          np.eye(128, dtype=np.float32)[:, None, None, :])      # [k,8,9,m]
    return dict(
        wqT=np.ascontiguousarray(wq2.T).astype(bf),
        wkT=np.ascontiguousarray(kw.T).astype(bf),
        wvT=np.ascontiguousarray(vw.T).astype(bf),
        wpT=np.ascontiguousarray(pw.T).astype(bf),
        qb2=np.ascontiguousarray(qb2v.reshape(16, 128).T).astype(np.float32),
        kb2=np.ascontiguousarray(kb.reshape(2, 128).T).astype(np.float32),
        vb2=np.ascontiguousarray(vb.reshape(8, 128).T).astype(np.float32),
        vbf=vb.astype(np.float32),
        bias1=np.ascontiguousarray(
            b1.transpose(1, 0, 2).reshape(196, 1568)).astype(bf),
        th2c=th2w.reshape(64).astype(np.float32),
        th2bb=th2b.astype(np.float32),
        vlb2=np.ascontiguousarray(vlb.reshape(8, 128).T).astype(np.float32),
        pb2=np.ascontiguousarray(pb.reshape(3, 128).T).astype(np.float32),
        dwT=np.ascontiguousarray(dw.reshape(128, 72 * 128)).astype(bf),
    )


# ---------------------------------------------------------------------------
# Bass/Tile kernel: one core, `bpc` samples
# ---------------------------------------------------------------------------

def _build_bass_kernel(bpc):
    import concourse.bass as bass
    import concourse.tile as tile
    from concourse import mybir
    from concourse.bass2jax import bass_jit
    from concourse.masks import make_identity

    F32 = mybir.dt.float32
    BF16 = mybir.dt.bfloat16

    @bass_jit
    def attn_kernel(nc: bass.Bass, x, wqT, wkT, wvT, wpT, qb2, kb2, vb2,
                    vbf, bias1, th2c, th2bb, vlb2, pb2, dwT):
        y = nc.dram_tensor("y", [bpc, DIM, N], BF16, kind="ExternalOutput")
        with tile.TileContext(nc) as tc, ExitStack() as ctx:
            wp_ = ctx.enter_context(tc.tile_pool(name="wts", bufs=1))
            xin = ctx.enter_context(tc.tile_pool(name="xin", bufs=3))
            act = ctx.enter_context(tc.tile_pool(name="act", bufs=2))
            sml = ctx.enter_context(tc.tile_pool(name="sml", bufs=3))
            ps_ = ctx.enter_context(tc.tile_pool(name="ps", bufs=5, space="PSUM"))
            pst_ = ctx.enter_context(tc.tile_pool(name="pst", bufs=3, space="PSUM"))

            # ---- weights -> SBUF (once per exec) ----
            wq_sb = wp_.tile([128, 3 * 2048], BF16)
            wk_sb = wp_.tile([128, 3 * 256], BF16)
            wv_sb = wp_.tile([128, 3 * 1024], BF16)
            for kc in range(3):
                nc.sync.dma_start(out=wq_sb[:, kc * 2048:(kc + 1) * 2048],
                                  in_=wqT[kc * 128:(kc + 1) * 128, :])
                nc.sync.dma_start(out=wk_sb[:, kc * 256:(kc + 1) * 256],
                                  in_=wkT[kc * 128:(kc + 1) * 128, :])
                nc.sync.dma_start(out=wv_sb[:, kc * 1024:(kc + 1) * 1024],
                                  in_=wvT[kc * 128:(kc + 1) * 128, :])
            wp_sb = wp_.tile([128, 8 * 384], BF16)
            for dc in range(8):
                nc.sync.dma_start(out=wp_sb[:, dc * 384:(dc + 1) * 384],
                                  in_=wpT[dc * 128:(dc + 1) * 128, :])
            b1_sb = []
            for ic, (nsl, nn) in enumerate(NCH):
                t = wp_.tile([128, 1568], BF16, tag=f"b1_{ic}")
                nc.sync.dma_start(out=t[:nn, :], in_=bias1[nsl:nsl + nn, :])
                b1_sb.append(t)
            dw_sb = wp_.tile([128, 72 * 128], BF16)
            nc.sync.dma_start(out=dw_sb, in_=dwT[:, :])
            vbf_sb = wp_.tile([128, 1024], F32)
            nc.sync.dma_start(out=vbf_sb, in_=vbf[None, :].to_broadcast((128, 1024)))
            th2c_sb = wp_.tile([128, 64], F32)
            nc.sync.dma_start(out=th2c_sb, in_=th2c[None, :].to_broadcast((128, 64)))
            th2b_sb = wp_.tile([128, 8], F32)
            nc.sync.dma_start(out=th2b_sb, in_=th2bb[None, :].to_broadcast((128, 8)))
            qb_sb = wp_.tile([128, 16], F32)
            nc.sync.dma_start(out=qb_sb, in_=qb2[:, :])
            kb_sb = wp_.tile([128, 2], F32)
            nc.sync.dma_start(out=kb_sb, in_=kb2[:, :])
            vb_sb = wp_.tile([128, 8], F32)
            nc.sync.dma_start(out=vb_sb, in_=vb2[:, :])
            vlb_sb = wp_.tile([128, 8], F32)
            nc.sync.dma_start(out=vlb_sb, in_=vlb2[:, :])
            pb_sb = wp_.tile([128, 3], F32)
            nc.sync.dma_start(out=pb_sb, in_=pb2[:, :])
            ident = wp_.tile([128, 128], BF16)
            make_identity(nc, ident)

            AL = mybir.AluOpType
            AF = mybir.ActivationFunctionType

            for s in range(bpc):
                x_sb = xin.tile([128, 3, 196], BF16, tag="x")
                for kc in range(3):
                    nc.sync.dma_start(out=x_sb[:, kc, :],
                                      in_=x[s, kc * 128:(kc + 1) * 128, :])
                # ---- q' projection (th1+SCALE prefolded): [2048, n] ----
                q_sb = act.tile([128, 16 * 196], BF16, tag="q")
                for mo in range(16):
                    ps = ps_.tile([128, 512], F32, tag="mm")
                    pq = ps[:, :196]
                    for kc in range(3):
                        nc.tensor.matmul(
                            pq, wq_sb[:, kc * 2048 + mo * 128:kc * 2048 + (mo + 1) * 128],
                            x_sb[:, kc, :], start=(kc == 0), stop=(kc == 2))
                    nc.vector.tensor_scalar_add(
                        q_sb[:, mo * 196:(mo + 1) * 196], pq, qb_sb[:, mo:mo + 1])
                # ---- k projection: [256, m] ----
                k_sb = act.tile([128, 2 * 196], BF16, tag="k")
                for mo in range(2):
                    ps = ps_.tile([128, 512], F32, tag="mm")
                    pk = ps[:, :196]
                    for kc in range(3):
                        nc.tensor.matmul(
                            pk, wk_sb[:, kc * 256 + mo * 128:kc * 256 + (mo + 1) * 128],
                            x_sb[:, kc, :], start=(kc == 0), stop=(kc == 2))
                    nc.vector.tensor_scalar_add(
                        k_sb[:, mo * 196:(mo + 1) * 196], pk, kb_sb[:, mo:mo + 1])
                # ---- vT: [m(128+68), dh1024] (attention V incl bias) ----
                vT_sb = act.tile([128, 2 * 1024], BF16, tag="vT")
                for mc, (msl, mm) in enumerate(NCH):
                    for dsv in range(2):
                        ps = ps_.tile([128, 512], F32, tag="mm")
                        for kc in range(3):
                            nc.tensor.matmul(
                                ps[:mm, :], x_sb[:, kc, msl:msl + mm],
                                wv_sb[:, kc * 1024 + dsv * 512:kc * 1024 + (dsv + 1) * 512],
                                start=(kc == 0), stop=(kc == 2))
                        nc.vector.tensor_tensor(
                            vT_sb[:mm, mc * 1024 + dsv * 512:mc * 1024 + (dsv + 1) * 512],
                            ps[:mm, :], vbf_sb[:mm, dsv * 512:(dsv + 1) * 512], AL.add)
                # ---- v4 -> zero-padded vp tiles [d, 16x16] (bias here) ----
                vp_sb = act.tile([128, 8, 16, 16], BF16, tag="vp")
                for dc in range(8):
                    ps = ps_.tile([128, 512], F32, tag="mm")
                    pv = ps[:, :196]
                    for kc in range(3):
                        nc.tensor.matmul(
                            pv, wv_sb[:, kc * 1024 + dc * 128:kc * 1024 + (dc + 1) * 128],
                            x_sb[:, kc, :], start=(kc == 0), stop=(kc == 2))
                    nc.vector.memset(vp_sb[:, dc, :, :], 0.0)
                    nc.vector.tensor_scalar_add(
                        vp_sb[:, dc, 1:15, 1:15],
                        pv.rearrange("p (h w) -> p h w", h=14), vb_sb[:, dc:dc + 1])
                # ---- logits (incl bias1 via identity matmul), exp, sums ----
                exp_sb = act.tile([128, 2 * 1568], BF16, tag="exp")
                S_sb = sml.tile([128, 16], F32, tag="S")
                R_sb = sml.tile([128, 16], F32, tag="R")
                for o in range(8):
                    for ic, (nsl, nn) in enumerate(NCH):
                        ps = ps_.tile([128, 512], F32, tag="mm")
                        pl = ps[:nn, :196]
                        nc.tensor.matmul(pl, ident[:nn, :nn],
                                         b1_sb[ic][:nn, o * 196:(o + 1) * 196],
                                         start=True, stop=False)
                        for kc in range(2):
                            nc.tensor.matmul(
                                pl,
                                q_sb[:, (2 * o + kc) * 196 + nsl:(2 * o + kc) * 196 + nsl + nn],
                                k_sb[:, kc * 196:(kc + 1) * 196],
                                start=False, stop=(kc == 1))
                        nc.scalar.activation(
                            exp_sb[:nn, ic * 1568 + o * 196:ic * 1568 + (o + 1) * 196],
                            pl, AF.Exp,
                            accum_out=S_sb[:nn, ic * 8 + o:ic * 8 + o + 1])
                for ic, (nsl, nn) in enumerate(NCH):
                    nc.vector.reciprocal(R_sb[:nn, ic * 8:(ic + 1) * 8],
                                         S_sb[:nn, ic * 8:(ic + 1) * 8])
                # ---- normalize ----
                pn_sb = act.tile([128, 2 * 1568], BF16, tag="pn")
                for o in range(8):
                    for ic, (nsl, nn) in enumerate(NCH):
                        sl = slice(ic * 1568 + o * 196, ic * 1568 + (o + 1) * 196)
                        nc.vector.tensor_scalar_mul(
                            pn_sb[:nn, sl], exp_sb[:nn, sl],
                            R_sb[:nn, ic * 8 + o:ic * 8 + o + 1])
                # ---- per head: th2 mix (FMA chain), transpose, out ----
                xr_sb = act.tile([128, 8 * 196], BF16, tag="xr")
                for h in range(8):
                    ph = sml.tile([128, 2 * 196], BF16, tag="ph")
                    tmp = sml.tile([128, 2 * 2 * 196], BF16, tag="mixtmp")
                    for ic, (nsl, nn) in enumerate(NCH):
                        acc = [tmp[:nn, (2 * ic) * 196:(2 * ic + 1) * 196],
                               tmp[:nn, (2 * ic + 1) * 196:(2 * ic + 2) * 196]]
                        for o in range(8):
                            pno = pn_sb[:nn, ic * 1568 + o * 196:ic * 1568 + (o + 1) * 196]
                            dst = (ph[:nn, ic * 196:(ic + 1) * 196] if o == 7
                                   else acc[(o + 1) % 2])
                            if o == 0:
                                nc.vector.tensor_scalar(
                                    dst, pno, th2c_sb[:nn, h * 8:h * 8 + 1],
                                    th2b_sb[:nn, h:h + 1], AL.mult, AL.add)
                            else:
                                nc.vector.scalar_tensor_tensor(
                                    dst, pno, th2c_sb[:nn, h * 8 + o:h * 8 + o + 1],
                                    acc[o % 2], AL.mult, AL.add)
                    pt_sb = sml.tile([128, 2 * 196], BF16, tag="pt")
                    for mc, (msl, mm) in enumerate(NCH):
                        pst = pst_.tile([128, 512], BF16, tag="mmt")
                        for ic, (nsl, nn) in enumerate(NCH):
                            nc.tensor.transpose(
                                pst[:mm, nsl:nsl + nn],
                                ph[:nn, ic * 196 + msl:ic * 196 + msl + mm],
                                ident[:nn, :nn])
                        nc.scalar.copy(pt_sb[:mm, mc * 196:(mc + 1) * 196],
                                       pst[:mm, :196])
                    # out^T_h[d, n]: attention-out + 9 depthwise taps in
                    # one PSUM accumulation group, then fused ReLU+bias
                    pso = ps_.tile([128, 512], F32, tag="mm")
                    po = pso[:, :196]
                    for mc, (msl, mm) in enumerate(NCH):
                        nc.tensor.matmul(
                            po, vT_sb[:mm, mc * 1024 + h * 128:mc * 1024 + (h + 1) * 128],
                            pt_sb[:mm, mc * 196:(mc + 1) * 196],
                            start=(mc == 0), stop=False)
                    for t in range(9):
                        dy, dx = t // 3, t % 3
                        nc.tensor.matmul(
                            po, dw_sb[:, (h * 9 + t) * 128:(h * 9 + t + 1) * 128],
                            vp_sb[:, h, dy:dy + 14, dx:dx + 14],
                            start=False, stop=(t == 8))
                    nc.scalar.activation(xr_sb[:, h * 196:(h + 1) * 196], po,
                                         AF.Relu, bias=vlb_sb[:, h:h + 1])
                # ---- output projection ----
                y_sb = xin.tile([128, 3 * 196], BF16, tag="y")
                for oc in range(3):
                    ps = ps_.tile([128, 512], F32, tag="mm")
                    py = ps[:, :196]
                    for dc in range(8):
                        nc.tensor.matmul(
                            py, wp_sb[:, dc * 384 + oc * 128:dc * 384 + (oc + 1) * 128],
                            xr_sb[:, dc * 196:(dc + 1) * 196],
                            start=(dc == 0), stop=(dc == 7))
                    nc.vector.tensor_scalar_add(
                        y_sb[:, oc * 196:(oc + 1) * 196], py, pb_sb[:, oc:oc + 1])
                    nc.sync.dma_start(out=y[s, oc * 128:(oc + 1) * 128, :],
                                      in_=y_sb[:, oc * 196:(oc + 1) * 196])
        return (y,)

    return attn_kernel


# ---------------------------------------------------------------------------
# jnp compute fallback (shard body)
# ---------------------------------------------------------------------------

def _block(x, q_w, q_b, k_w, k_b, v_w, v_b, vl_w, vl_b,
           th1_w, th1_b, th2_w, th2_b, proj_w, proj_b, bias):
    import jax
    import jax.numpy as jnp
    b = x.shape[0]
    xf = x.astype(jnp.float32)
    q = jnp.einsum('oc,bcn->bon', q_w, xf) + q_b[:, None]
    k = jnp.einsum('oc,bcn->bon', k_w, xf) + k_b[:, None]
    v = jnp.einsum('oc,bcn->bon', v_w, xf) + v_b[:, None]
    v4 = v.reshape(b, DH, RES, RES)
    vp = jnp.pad(v4, ((0, 0), (0, 0), (1, 1), (1, 1)))
    vloc = vl_b[None, :, None, None]
    for dy in range(3):
        for dx in range(3):
            vloc = vloc + vl_w[:, dy, dx][None, :, None, None] * \
                vp[:, :, dy:dy + RES, dx:dx + RES]
    qh = q.reshape(b, HEADS, KEY_DIM, N)
    kh = k.reshape(b, HEADS, KEY_DIM, N)
    attn = jnp.einsum('bhcn,bhcm->bhnm', qh, kh) * SCALE + bias[None]
    attn = jnp.einsum('oi,binm->bonm', th1_w, attn) + th1_b[None, :, None, None]
    attn = jax.nn.softmax(attn, axis=-1)
    attn = jnp.einsum('oi,binm->bonm', th2_w, attn) + th2_b[None, :, None, None]
    vh = v.reshape(b, HEADS, D, N)
    out = jnp.einsum('bhnm,bhdm->bhdn', attn, vh)
    x_out = jax.nn.relu(out.reshape(b, DH, N) + vloc.reshape(b, DH, N))
    y = jnp.einsum('oc,bcn->bon', proj_w, x_out) + proj_b[:, None]
    return y.astype(jnp.bfloat16)


def _block_np(x, q_w, q_b, k_w, k_b, v_w, v_b, vl_w, vl_b,
              th1_w, th1_b, th2_w, th2_b, proj_w, proj_b, bias):
    b = x.shape[0]
    xf = x.reshape(b, DIM, N)
    q = np.einsum('oc,bcn->bon', q_w, xf) + q_b[:, None]
    k = np.einsum('oc,bcn->bon', k_w, xf) + k_b[:, None]
    v = np.einsum('oc,bcn->bon', v_w, xf) + v_b[:, None]
    v4 = v.reshape(b, DH, RES, RES)
    vp = np.pad(v4, ((0, 0), (0, 0), (1, 1), (1, 1)))
    vloc = np.broadcast_to(vl_b[None, :, None, None], v4.shape).copy()
    for dy in range(3):
        for dx in range(3):
            vloc += vl_w[:, dy, dx][None, :, None, None] * \
                vp[:, :, dy:dy + RES, dx:dx + RES]
    qh = q.reshape(b, HEADS, KEY_DIM, N)
    kh = k.reshape(b, HEADS, KEY_DIM, N)
    attn = np.einsum('bhcn,bhcm->bhnm', qh, kh) * SCALE + bias[None]
    attn = np.einsum('oi,binm->bonm', th1_w, attn) + th1_b[None, :, None, None]
    attn = attn - attn.max(-1, keepdims=True)
    np.exp(attn, out=attn)
    attn /= attn.sum(-1, keepdims=True)
    attn = np.einsum('oi,binm->bonm', th2_w, attn) + th2_b[None, :, None, None]
    vh = v.reshape(b, HEADS, D, N)
    out = np.einsum('bhnm,bhdm->bhdn', attn, vh)
    x_out = np.maximum(out.reshape(b, DH, RES, RES) + vloc, 0.0)
    y = np.einsum('oc,bcn->bon', proj_w, x_out.reshape(b, DH, N)) + proj_b[:, None]
    return y.reshape(b, DIM, RES, RES).astype(np.float32)


# ---------------------------------------------------------------------------
# dispatch
# ---------------------------------------------------------------------------

def kernel(x, **kw):
    x = np.asarray(x, np.float32)
    wargs = _prep_weights(**kw)
    try:
        return _run_bass(x, wargs)
    except Exception:
        import traceback
        traceback.print_exc()
        try:
            return _run_device(x, wargs)
        except Exception:
            traceback.print_exc()
            return _block_np(x, *wargs)


def _jax_setup():
    import os
    os.environ.setdefault("JAX_COMPILATION_CACHE_DIR", "/tmp/jax_comp_cache")
    import jax
    jax.config.update("jax_compilation_cache_dir",
                      os.environ["JAX_COMPILATION_CACHE_DIR"])
    jax.config.update("jax_persistent_cache_min_entry_size_bytes", -1)
    jax.config.update("jax_persistent_cache_min_compile_time_secs", 0)
    from jax.sharding import Mesh, PartitionSpec as P, NamedSharding
    devs = jax.devices()[:NCORES]
    mesh = Mesh(np.asarray(devs), ("b",))
    return jax, mesh, P, NamedSharding


def _run_bass(x, wargs):
    import ml_dtypes
    sched = tuple(SCHED) if (INT8_IN and SCHED) else (CBPC,) * NCHUNK
    assert sum(sched) == BPC
    fp = ("bass", sched) + tuple(float(a.sum()) for a in wargs)
    if _cache.get("bass_fp") != fp:
        jax, mesh, P, NamedSharding = _jax_setup()
        import jax.numpy as jnp
        bw = _prep_bass_weights(*wargs)
        sh_r = NamedSharding(mesh, P())
        sh_b = NamedSharding(mesh, P("b"))
        wdev = tuple(jax.device_put(bw[k], sh_r) for k in BASS_WEIGHT_ORDER)
        nw = len(BASS_WEIGHT_ORDER)

        def _quant_body(yb):
            yf = yb.astype(jnp.float32)
            am = jnp.max(jnp.abs(yf), axis=-1) + 1e-8
            sc = am / 127.0
            yq = jnp.round(yf / sc[:, :, None]).astype(jnp.int8)
            return yq, sc

        def _dequant_body(xq, xsc):
            return (xq.astype(jnp.float32) *
                    xsc[:, :, None]).astype(jnp.bfloat16)

        # one (bass, quant, dequant) jit triple per distinct chunk size.
        # The neuronx_cc hook requires the bass_exec custom call to be
        # the ONLY op in its module, so quant/dequant are separate jits.
        fns = {}
        for bpc in sorted(set(sched)):
            kern = _build_bass_kernel(bpc)
            f = jax.jit(jax.shard_map(
                lambda xb, *w, _k=kern: _k(xb, *w)[0], mesh=mesh,
                in_specs=(P("b"),) + (P(),) * nw,
                out_specs=P("b"), check_vma=False))
            fq = jax.jit(jax.shard_map(
                _quant_body, mesh=mesh, in_specs=(P("b"),),
                out_specs=(P("b"), P("b")), check_vma=False))
            fdq = jax.jit(jax.shard_map(
                _dequant_body, mesh=mesh, in_specs=(P("b"), P("b")),
                out_specs=P("b"), check_vma=False))
            # warm so timed calls skip tracing/compiling
            zx = np.zeros((NCORES * bpc, DIM, N), ml_dtypes.bfloat16)
            if INT8_IN:
                zq, zs = fq(f(fdq(
                    jax.device_put(zx.astype(np.int8), sh_b),
                    jax.device_put(
                        np.ones((NCORES * bpc, DIM), np.float32),
                        sh_b)), *wdev))
            else:
                zq, zs = fq(f(jax.device_put(zx, sh_b), *wdev))
            zq.block_until_ready()
            fns[bpc] = (f, fq, fdq)
        _cache.update(bass_fns=fns, bass_w=wdev, bass_fp=fp, bass_sh=sh_b)

    fns, wdev, sh_b = _cache["bass_fns"], _cache["bass_w"], _cache["bass_sh"]
    import jax
    xr = x.reshape(NCORES, BPC, DIM, N)
    offs = [0]
    for bpc in sched:
        offs.append(offs[-1] + bpc)

    def _quant_chunk(c):
        xc = np.ascontiguousarray(
            xr[:, offs[c]:offs[c + 1]]).reshape(-1, DIM, N)
        am = np.abs(xc).max(axis=-1) + 1e-8
        sc = (am / 127.0).astype(np.float32)
        np.rint(xc * (1.0 / sc)[:, :, None], out=xc)
        return xc.astype(np.int8), sc

    hs = []
    if INT8_IN:
        if QUANT_THREADS:
            from concurrent.futures import ThreadPoolExecutor
            ex = _cache.get("ex")
            if ex is None:
                ex = _cache["ex"] = ThreadPoolExecutor(QUANT_THREADS)
            futs = [ex.submit(_quant_chunk, c) for c in range(len(sched))]
            chunks = (futs[c].result() for c in range(len(sched)))
        else:
            chunks = (_quant_chunk(c) for c in range(len(sched)))
        for bpc, (xq, xsc) in zip(sched, chunks):
            f, fq, fdq = fns[bpc]
            xb = fdq(jax.device_put(xq, sh_b), jax.device_put(xsc, sh_b))
            yq, sc = fq(f(xb, *wdev))
            yq.copy_to_host_async()
            sc.copy_to_host_async()
            hs.append((yq, sc))
    else:
        for c, bpc in enumerate(sched):
            f, fq, fdq = fns[bpc]
            xc = np.ascontiguousarray(
                xr[:, offs[c]:offs[c + 1]]).reshape(-1, DIM, N)
            yq, sc = fq(f(jax.device_put(
                xc.astype(ml_dtypes.bfloat16), sh_b), *wdev))
            yq.copy_to_host_async()
            sc.copy_to_host_async()
            hs.append((yq, sc))
    out = np.empty((NCORES, BPC, DIM, N), np.float32)
    for c, (yq, sc) in enumerate(hs):
        yqh = np.asarray(yq).astype(np.float32)
        sch = np.asarray(sc)
        out[:, offs[c]:offs[c + 1]] = \
            (yqh * sch[:, :, None]).reshape(NCORES, sched[c], DIM, N)
    return out.reshape(B, DIM, RES, RES)


def _run_device(x, wargs):
    import ml_dtypes
    fp = ("jnp",) + tuple(float(a.sum()) for a in wargs)
    if _cache.get("fp") != fp:
        jax, mesh, P, NamedSharding = _jax_setup()
        f = jax.jit(jax.shard_map(
            _block, mesh=mesh,
            in_specs=(P("b"),) + (P(),) * len(wargs),
            out_specs=P("b"), check_vma=False))
        sh_r = NamedSharding(mesh, P())
        wdev = tuple(jax.device_put(w, sh_r) for w in wargs)
        _cache.update(f=f, wdev=wdev, fp=fp,
                      sh_b=NamedSharding(mesh, P("b")))
    import jax
    xb = x.reshape(B, DIM, N).astype(ml_dtypes.bfloat16)
    xd = jax.device_put(xb, _cache["sh_b"])
    y = _cache["f"](xd, *_cache["wdev"])
    return np.asarray(y).astype(np.float32).reshape(B, DIM, RES, RES)
